# revision 1
# baseline (speedup 1.0000x reference)
"""Trainium2 Bass kernel for nn_ListenerModel (scatter_memory).

Strategy: pure data-parallel over batch (B=64 -> 8 rows/core), weights
replicated.  All matmuls are arranged so both operands load in natural
(row-major) layout; the big L=512-wide matmuls keep features on the
partition dim ([feat, L] outputs) so the chain
reps@W_emb -> @W_mm -> @W_a1 -> scores never needs an on-device
transpose of a large tensor.  Host pre-transposes reps / vc / sep once.
float32r operands get full PE rate at N=512 (plain fp32 is 4x slower).
DMAs are batched into multi-chunk 3D transfers to keep the Sync
sequencer's DIRECT2D descriptor generation off the critical path.
"""

import numpy as np
from contextlib import ExitStack

import concourse.bass as bass
import concourse.mybir as mybir
from concourse import bacc, tile
from concourse.bass_utils import run_bass_kernel_spmd

NCORES = 8
B, L, S, H = 64, 512, 6, 8
EMBED, HID, IMG, ATT = 1024, 512, 2048, 256
SIMG = S * IMG          # 12288
BC = B // NCORES        # 8 batch rows per core
BS = BC * S             # 48 (b,s) rows per core
BSH = BS * H            # 384
P = 128
FP = mybir.dt.float32
FPR = mybir.dt.float32r

KE = EMBED // P         # 8  k-chunks for EMBED contraction
KH = HID // P           # 4  k-chunks for HID contraction
KA = ATT // P           # 2  k-chunks for ATT contraction
KV = SIMG // P          # 96 k-chunks for the visual-context matmul
KI = IMG // P           # 16 k-chunks for separate-image projection
KBH = BSH // P          # 3  k-chunks for history averaging
NHT = HID // P          # 4  hid tiles
NAT = ATT // P          # 2  att tiles

WVB = 2                 # W_vis chunks per DMA
RPB = 4                 # reps chunks per DMA


def build_nc():
    nc = bacc.Bacc(None)

    # ---- DRAM I/O (per-core shapes); FPR = feeds a float32r matmul ----
    # 3D DRAM views are pre-chunked on the host: [n_chunks, 128, width]
    d_repsT = nc.dram_tensor("repsT", [BC, KE, P, L], FPR, kind="ExternalInput")
    d_vcT = nc.dram_tensor("vcT", [KV, P, BC], FPR, kind="ExternalInput")
    d_sepT = nc.dram_tensor("sepT", [KI, P, BS], FPR, kind="ExternalInput")
    d_hist = nc.dram_tensor("histf", [KBH, P, EMBED], FP, kind="ExternalInput")
    d_validW = nc.dram_tensor("validW", [KBH, P, BS], FP, kind="ExternalInput")
    d_Wvis = nc.dram_tensor("Wvis", [KV, P, HID], FPR, kind="ExternalInput")
    d_Wemb = nc.dram_tensor("Wemb", [KE, P, HID], FPR, kind="ExternalInput")
    d_Wmm = nc.dram_tensor("Wmm", [2 * KH, P, HID], FPR, kind="ExternalInput")
    d_Wsep = nc.dram_tensor("Wsep", [KI, P, HID], FPR, kind="ExternalInput")
    d_Wa1 = nc.dram_tensor("Wa1", [KH, P, ATT], FPR, kind="ExternalInput")
    d_Wa2 = nc.dram_tensor("Wa2", [KA, P, 1], FPR, kind="ExternalInput")
    d_bvis = nc.dram_tensor("bvis_row", [1, HID], FPR, kind="ExternalInput")
    d_bsep = nc.dram_tensor("bsep_row", [1, HID], FPR, kind="ExternalInput")
    d_bemb_row = nc.dram_tensor("bemb_row", [1, HID], FPR, kind="ExternalInput")
    d_ones = nc.dram_tensor("ones_row", [1, P], FPR, kind="ExternalInput")
    d_bemb_col = nc.dram_tensor("bemb_col", [NHT, P, 1], FP, kind="ExternalInput")
    d_bmm_col = nc.dram_tensor("bmm_col", [NHT, P, 1], FP, kind="ExternalInput")
    d_ba1_col = nc.dram_tensor("ba1_col", [NAT, P, 1], FP, kind="ExternalInput")
    d_mask = nc.dram_tensor("mask_row", [BC, L], FP, kind="ExternalInput")
    d_hh = nc.dram_tensor("hh_col", [BS, 1], FP, kind="ExternalInput")
    d_diagT = nc.dram_tensor("diagT", [BC, BS], FPR, kind="ExternalInput")
    d_ident = nc.dram_tensor("ident", [P, P], FP, kind="ExternalInput")
    d_out = nc.dram_tensor("out", [BS, 1], FP, kind="ExternalOutput")

    AFT = mybir.ActivationFunctionType
    AX = mybir.AxisListType

    with ExitStack() as ctx:
        tc = ctx.enter_context(tile.TileContext(nc))
        wres = ctx.enter_context(tc.tile_pool(name="wres", bufs=1))
        repsp = ctx.enter_context(tc.tile_pool(name="repsp", bufs=4))
        wvp = ctx.enter_context(tc.tile_pool(name="wvp", bufs=4))
        wsp = ctx.enter_context(tc.tile_pool(name="wsp", bufs=2))
        mm1p = ctx.enter_context(tc.tile_pool(name="mm1p", bufs=16))
        mm2p = ctx.enter_context(tc.tile_pool(name="mm2p", bufs=6))
        atthp = ctx.enter_context(tc.tile_pool(name="atthp", bufs=4))
        tmpp = ctx.enter_context(tc.tile_pool(name="tmpp", bufs=2))
        smp = ctx.enter_context(tc.tile_pool(name="smp", bufs=1))
        psA = ctx.enter_context(tc.tile_pool(name="psA", bufs=6, space="PSUM"))
        psB = ctx.enter_context(tc.tile_pool(name="psB", bufs=2, space="PSUM"))

        def wtile(shape, tag, dt=FP):
            return wres.tile(shape, dt, tag=tag, name=tag)

        def load(dst, src):
            nc.sync.dma_start(out=dst, in_=src)

        def body():
            # ---- streaming loads emitted first: W_vis + vcT get queue
            # priority so ctxmm unblocks as early as possible ----
            vct = wtile([P, KV, BC], "vct", FPR)          # all 96 chunks
            load(vct, d_vcT.rearrange("k p b -> p k b"))
            wv_tiles = []
            for i in range(KV // WVB):
                wv = wvp.tile([P, WVB, HID], FPR, tag="wv", name="wv")
                load(wv, d_Wvis[i * WVB:(i + 1) * WVB].rearrange(
                    "k p h -> p k h"))
                wv_tiles.append(wv)

            # ---- constants / small tensors ----
            ones = wtile([1, P], "ones", FPR)
            load(ones, d_ones[:, :])
            ident = wtile([P, P], "ident")
            load(ident, d_ident[:, :])
            hh_sb = wtile([BS, 1], "hh")
            load(hh_sb, d_hh[:, :])
            diagT_sb = wtile([BC, BS], "diagT", FPR)
            load(diagT_sb, d_diagT[:, :])
            bvis_sb = wtile([1, HID], "bvis", FPR)
            load(bvis_sb, d_bvis[:, :])
            bsep_sb = wtile([1, HID], "bsep", FPR)
            load(bsep_sb, d_bsep[:, :])
            bembr_sb = wtile([1, HID], "bembr", FPR)
            load(bembr_sb, d_bemb_row[:, :])
            bembc_sb = wtile([P, NHT], "bembc")
            load(bembc_sb, d_bemb_col.rearrange("h p one -> p (h one)"))
            ba1c_sb = wtile([P, NAT], "ba1c")
            load(ba1c_sb, d_ba1_col.rearrange("a p one -> p (a one)"))
            bmmc_sb = wtile([P, NHT], "bmmc")
            load(bmmc_sb, d_bmm_col.rearrange("h p one -> p (h one)"))
            wa2_sb = wtile([P, KA], "wa2", FPR)
            load(wa2_sb, d_Wa2.rearrange("k p one -> p (k one)"))
            validW_sb = wtile([P, KBH, BS], "validW")
            load(validW_sb, d_validW.rearrange("k p s -> p k s"))

            # ---- resident weights (single batched DMAs) ----
            wemb = wtile([P, KE, HID], "wemb", FPR)
            load(wemb, d_Wemb.rearrange("k p h -> p k h"))
            wmm = wtile([P, 2 * KH, HID], "wmm", FPR)
            load(wmm, d_Wmm.rearrange("k p h -> p k h"))
            wa1 = wtile([P, KH, ATT], "wa1", FPR)
            load(wa1, d_Wa1.rearrange("k p h -> p k h"))
            sepT_sb = wtile([P, KI, BS], "sepT", FPR)
            load(sepT_sb, d_sepT.rearrange("k p s -> p k s"))
            histf_sb = wtile([P, KBH, EMBED], "histf")
            load(histf_sb, d_hist.rearrange("k p e -> p k e"))

            # ---- visual context projection, interleaved with mm1 ----
            vc_psum = psB.tile([BC, HID], FP, tag="B", name="vc_psum")
            mm1_sb = {}

            def emit_vc_group(i):
                for j in range(WVB):
                    k = i * WVB + j
                    nc.tensor.matmul(vc_psum[:, :], vct[:, k, :],
                                     wv_tiles[i][:, j, :],
                                     start=(k == 0), stop=False)

            def emit_mm1_b(b):
                # mm1T[b]: [hid, L] = (W_emb.T @ reps[b].T), relu(+b_emb)
                rt = []
                for i in range(KE // RPB):
                    t = repsp.tile([P, RPB, L], FPR, tag="reps", name="rt")
                    load(t, d_repsT[b, i * RPB:(i + 1) * RPB].rearrange(
                        "k p l -> p k l"))
                    rt.append(t)
                for h in range(NHT):
                    ps = psA.tile([P, L], FP, tag="A", name="mm1ps")
                    for k in range(KE):
                        nc.tensor.matmul(
                            ps[:, :],
                            wemb[:, k, h * P:(h + 1) * P],
                            rt[k // RPB][:, k % RPB, :],
                            start=(k == 0), stop=(k == KE - 1))
                    t = mm1p.tile([P, L], FPR, tag="mm1", name=f"mm1_{b}_{h}")
                    nc.scalar.activation(t, ps[:, :], AFT.Relu,
                                         bias=bembc_sb[:, h:h + 1])
                    mm1_sb[(b, h)] = t

            # 48 vc chunk-groups interleaved with mm1 for b=0..3
            gpb = (KV // WVB) // 4  # 12 groups per b
            for b in range(4):
                for i in range(b * gpb, (b + 1) * gpb):
                    emit_vc_group(i)
                emit_mm1_b(b)

            # bias matmul: ones[1,8].T @ b_vis[1,512] adds b_vis to all rows
            nc.tensor.matmul(vc_psum[:, :], ones[:, :BC], bvis_sb[:, :],
                             start=False, stop=True)
            ctx_sb = wtile([BC, HID], "ctx_sb")
            nc.scalar.activation(ctx_sb, vc_psum[:, :], AFT.Relu)

            # transpose ctx [8, 512] -> ctxT [512, 8] via PE (4x [8,128])
            ctxT_sb = [wtile([P, BC], f"ctxT{h}", FPR) for h in range(NHT)]
            for h in range(NHT):
                tp = psB.tile([P, BC], FP, tag="B", name="ctxT_ps")
                nc.tensor.transpose(tp[:, :], ctx_sb[:, h * P:(h + 1) * P],
                                    ident[:BC, :BC])
                nc.scalar.activation(ctxT_sb[h], tp[:, :], AFT.Identity)

            # ctxmmb[h2] = W_mm_bot.T @ ctxT + b_mm   [128, 8] per hid2 tile
            ctxmmb_sb = [wtile([P, BC], f"ctxmmb{h}") for h in range(NHT)]
            for h2 in range(NHT):
                ps = psB.tile([P, BC], FP, tag="B", name="ctxmm_ps")
                for k in range(KH):
                    nc.tensor.matmul(ps[:, :],
                                     wmm[:, KH + k, h2 * P:(h2 + 1) * P],
                                     ctxT_sb[k][:, :],
                                     start=(k == 0), stop=(k == KH - 1))
                nc.scalar.activation(ctxmmb_sb[h2], ps[:, :], AFT.Identity,
                                     bias=bmmc_sb[:, h2:h2 + 1])

            # ---- separate images projection: sep[48, 512] ----
            sep_ps = psB.tile([BS, HID], FP, tag="B", name="sep_ps")
            for i in range(KI // 4):
                ws = wsp.tile([P, 4, HID], FPR, tag="ws", name="ws")
                load(ws, d_Wsep[i * 4:(i + 1) * 4].rearrange("k p h -> p k h"))
                for j in range(4):
                    k = i * 4 + j
                    nc.tensor.matmul(sep_ps[:, :], sepT_sb[:, k, :],
                                     ws[:, j, :],
                                     start=(k == 0), stop=False)
            nc.tensor.matmul(sep_ps[:, :], ones[:, :BS], bsep_sb[:, :],
                             start=False, stop=True)
            sep_sb = wtile([BS, HID], "sep_sb")
            nc.vector.tensor_copy(sep_sb, sep_ps[:, :])

            # ---- history: havgT[e,48] via block-diag valid-weight matmul ----
            havgT_sb = [wtile([P, BS], f"havgT{e}", FPR) for e in range(KE)]
            for e in range(KE):
                ps = psB.tile([P, BS], FP, tag="B", name="havg_ps")
                for k in range(KBH):
                    nc.tensor.matmul(ps[:, :],
                                     histf_sb[:, k, e * P:(e + 1) * P],
                                     validW_sb[:, k, :],
                                     start=(k == 0), stop=(k == KBH - 1))
                nc.scalar.activation(havgT_sb[e], ps[:, :], AFT.Identity)

            # hist_add[48, 512] = relu(havg @ W_emb + b_emb)
            ha_ps = psB.tile([BS, HID], FP, tag="B", name="ha_ps")
            for e in range(KE):
                nc.tensor.matmul(ha_ps[:, :], havgT_sb[e][:, :],
                                 wemb[:, e, :],
                                 start=(e == 0), stop=False)
            nc.tensor.matmul(ha_ps[:, :], ones[:, :BS], bembr_sb[:, :],
                             start=False, stop=True)
            hadd_sb = wtile([BS, HID], "hadd_sb")
            nc.scalar.activation(hadd_sb, ha_ps[:, :], AFT.Relu)

            # sep_final = sep + hh * hist_add
            sepfin_sb = wtile([BS, HID], "sepfin_sb")
            nc.vector.tensor_scalar_mul(sepfin_sb, hadd_sb, hh_sb)
            nc.vector.tensor_add(sepfin_sb, sepfin_sb, sep_sb)

            # ---- per-b: mm2 -> mm3 -> scores -> softmax -> attended ----
            attT_sb = [wtile([P, BC], f"attT{h}") for h in range(NHT)]
            for b in range(BC):
                if b < 4:
                    emit_mm1_b(b + 4)
                # mm2T[b]: [hid2, L] = relu(Wmm_top.T @ mm1T[b] + ctxmm[:,b])
                mm2t = []
                for h2 in range(NHT):
                    ps = psA.tile([P, L], FP, tag="A", name="mm2ps")
                    for k in range(KH):
                        nc.tensor.matmul(ps[:, :],
                                         wmm[:, k, h2 * P:(h2 + 1) * P],
                                         mm1_sb[(b, k)][:, :],
                                         start=(k == 0), stop=(k == KH - 1))
                    t = mm2p.tile([P, L], FPR, tag="mm2", name="mm2t")
                    nc.scalar.activation(t, ps[:, :], AFT.Relu,
                                         bias=ctxmmb_sb[h2][:, b:b + 1])
                    mm2t.append(t)
                # mm3: atthT [att, L] = tanh(W_a1.T @ mm2T + b_a1)
                atth = []
                for a in range(NAT):
                    ps = psA.tile([P, L], FP, tag="A", name="mm3ps")
                    for k in range(KH):
                        nc.tensor.matmul(ps[:, :],
                                         wa1[:, k, a * P:(a + 1) * P],
                                         mm2t[k][:, :],
                                         start=(k == 0), stop=(k == KH - 1))
                    t = atthp.tile([P, L], FPR, tag="atth", name="atht")
                    nc.scalar.activation(t, ps[:, :], AFT.Tanh,
                                         bias=ba1c_sb[:, a:a + 1])
                    atth.append(t)
                # scores row [1, L] = W_a2.T @ atthT (+mask incl. b_a2)
                sc_ps = psA.tile([1, L], FP, tag="A", name="scps")
                for k in range(KA):
                    nc.tensor.matmul(sc_ps[:, :], wa2_sb[:, k:k + 1],
                                     atth[k][:, :],
                                     start=(k == 0), stop=(k == KA - 1))
                mrow = smp.tile([1, L], FP, tag="mrow", name="mrow")
                load(mrow, d_mask[b:b + 1, :])
                att_row = smp.tile([1, L], FP, tag="attrow", name="att_row")
                nc.vector.tensor_add(att_row, sc_ps[:, :], mrow)
                # softmax over L (free axis), exp in place
                negmax = smp.tile([1, 1], FP, tag="negmax", name="negmax")
                nc.vector.reduce_max(negmax, att_row, axis=AX.X, negate=True)
                esum = smp.tile([1, 1], FP, tag="esum", name="esum")
                nc.scalar.activation(att_row, att_row, AFT.Exp, bias=negmax,
                                     accum_out=esum)
                rec = smp.tile([1, 1], FP, tag="rec", name="rec")
                nc.vector.reciprocal(rec, esum)
                # normalize + fp32r-round in one ACT copy
                wrow = smp.tile([1, L], FPR, tag="wrow", name="wrow")
                nc.scalar.activation(wrow, att_row, AFT.Copy, scale=rec)
                # broadcast w row to [128, L] via PE ones-product
                wb_ps = psA.tile([P, L], FP, tag="A", name="wbps")
                nc.tensor.matmul(wb_ps[:, :], ones[:, :], wrow[:, :],
                                 start=True, stop=True)
                # attended[:, b] = sum_l mm2T * w  (DVE mul + reduce)
                for h2 in range(NHT):
                    tmp = tmpp.tile([P, L], FP, tag="tmpa", name="tmpa")
                    nc.vector.tensor_mul(tmp, mm2t[h2].bitcast(FP)[:, :],
                                         wb_ps[:, :])
                    nc.vector.reduce_sum(attT_sb[h2][:, b:b + 1], tmp,
                                         axis=AX.X)

            # ---- attended rows [8, 512] via PE transpose of attT tiles ----
            attrows_sb = wtile([BC, HID], "attrows", FPR)
            for h in range(NHT):
                tp = psB.tile([BC, P], FP, tag="B", name="attrow_ps")
                nc.tensor.transpose(tp[:, :], attT_sb[h][:, :], ident[:, :])
                nc.scalar.activation(attrows_sb[:, h * P:(h + 1) * P],
                                     tp[:, :], AFT.Identity)

            # broadcast to [48, 512]: diagT.T @ attrows
            ab_ps = psB.tile([BS, HID], FP, tag="B", name="ab_ps")
            nc.tensor.matmul(ab_ps[:, :], diagT_sb[:, :], attrows_sb[:, :],
                             start=True, stop=True)
            # dot: out[48] = sum_hid sep_final * attended_bcast
            prod = tmpp.tile([BS, HID], FP, tag="tmpa", name="prod")
            nc.vector.tensor_mul(prod, sepfin_sb, ab_ps[:, :])
            out_sb = wtile([BS, 1], "out_sb")
            nc.vector.reduce_sum(out_sb, prod, axis=AX.X)
            nc.sync.dma_start(out=d_out[:, :], in_=out_sb)

        body()

    nc.compile()
    return nc


_NC_CACHE = None


def kernel(reps, separate_imgs, visual_context, masks, hist, hist_len,
           W_vis, b_vis, W_emb, b_emb, W_mm, b_mm, W_sep, b_sep,
           W_a1, b_a1, W_a2, b_a2):
    global _NC_CACHE
    f32 = np.float32

    def chunk(a):
        """[K, W] -> [K//128, 128, W] view."""
        a = np.ascontiguousarray(a, f32)
        return a.reshape(a.shape[0] // P, P, a.shape[1])

    reps = np.asarray(reps, f32)
    separate_imgs = np.asarray(separate_imgs, f32)
    visual_context = np.asarray(visual_context, f32)
    hist = np.asarray(hist, f32)
    hist_len = np.asarray(hist_len, np.int32)
    masks = np.asarray(masks)

    repsT = np.ascontiguousarray(reps.transpose(0, 2, 1))        # [B, EMBED, L]
    vcT = np.ascontiguousarray(visual_context.T)                 # [SIMG, B]
    mask_row = np.where(masks[:, :, 0], f32(-1e30), f32(0.0)) + f32(b_a2[0])
    ident = np.eye(P, dtype=f32)

    shared = {
        "Wvis": chunk(W_vis),
        "Wemb": chunk(W_emb),
        "Wmm": chunk(W_mm),
        "Wsep": chunk(W_sep),
        "Wa1": chunk(W_a1),
        "Wa2": chunk(np.ascontiguousarray(W_a2, f32).reshape(ATT, 1)),
        "bvis_row": np.ascontiguousarray(b_vis, f32).reshape(1, HID),
        "bsep_row": np.ascontiguousarray(b_sep, f32).reshape(1, HID),
        "bemb_row": np.ascontiguousarray(b_emb, f32).reshape(1, HID),
        "bemb_col": np.ascontiguousarray(b_emb, f32).reshape(NHT, P, 1),
        "bmm_col": np.ascontiguousarray(b_mm, f32).reshape(NHT, P, 1),
        "ba1_col": np.ascontiguousarray(b_a1, f32).reshape(NAT, P, 1),
        "ones_row": np.ones((1, P), f32),
        "ident": ident,
        "diagT": np.repeat(np.eye(BC, dtype=f32), S, axis=1).reshape(BC, BS),
    }

    in_maps = []
    for c in range(NCORES):
        sl = slice(c * BC, (c + 1) * BC)
        hl = hist_len[sl].reshape(BS)                            # [48]
        hvalid = (np.arange(H)[None, :] < hl[:, None]).astype(f32)
        hvalid /= np.maximum(hl, 1).astype(f32)[:, None]         # [48, H]
        validW = np.zeros((BSH, BS), f32)
        for bs in range(BS):
            validW[bs * H:(bs + 1) * H, bs] = hvalid[bs]
        m = {
            "repsT": np.ascontiguousarray(repsT[sl]).reshape(BC, KE, P, L),
            "vcT": chunk(np.ascontiguousarray(vcT[:, sl])),
            "sepT": chunk(np.ascontiguousarray(
                separate_imgs[sl].reshape(BS, IMG).T)),
            "histf": chunk(hist[sl].reshape(BSH, EMBED)),
            "validW": chunk(validW),
            "mask_row": np.ascontiguousarray(mask_row[sl]),
            "hh_col": (hl > 0).astype(f32).reshape(BS, 1),
        }
        m.update(shared)
        in_maps.append(m)

    if _NC_CACHE is None:
        _NC_CACHE = build_nc()
    res = run_bass_kernel_spmd(_NC_CACHE, in_maps, list(range(NCORES)))
    out = np.concatenate([r["out"].reshape(BC, S, 1) for r in res.results],
                         axis=0)
    return out.astype(f32)


if __name__ == "__main__":
    pass



# revision 9
# speedup vs baseline: 1.8423x; 1.8423x over previous
"""Trainium2 Bass kernel for nn_ListenerModel (scatter_memory).

Pure data-parallel over batch (B=64 -> 8 rows/core), weights replicated.
All large streams (reps, W_vis, W_emb, W_mm, W_sep, W_a1, hist, ...) are
converted to bf16 on the host: the PE streams bf16 at the same 1 row/cycle
as float32r, so matmul time is unchanged while HBM traffic halves
(~53 MB -> ~27 MB per core).  Everything is host-prepacked into
[128, free] contiguous blocks so each DMA is 128 large contiguous
descriptors.  Program order keeps the PE hot: mm1 over all 8 batch rows
first with the W_vis/vc chunk-stream interleaved, then the per-batch
mm2 -> mm3 -> scores chain with the softmax/attended work pipelined one
batch behind so small-op latency never stalls the PE queue.  PSUM stays
fp32 throughout; softmax normalization is deferred to a single per-row
scale at the end (attended rows scaled by 1/esum via ACT scale column).
"""

import os
import numpy as np
import ml_dtypes
from contextlib import ExitStack

import concourse.bass as bass
import concourse.mybir as mybir
from concourse import bacc, tile
from concourse.bass_utils import run_bass_kernel_spmd

NCORES = 8
B, L, S, H = 64, 512, 6, 8
EMBED, HID, IMG, ATT = 1024, 512, 2048, 256
SIMG = S * IMG          # 12288
BC = B // NCORES        # 8 batch rows per core
BS = BC * S             # 48 (b,s) rows per core
BSH = BS * H            # 384
P = 128
FP = mybir.dt.float32
USE_BF16 = os.environ.get("KBF", "1") == "1"
BF = mybir.dt.bfloat16 if USE_BF16 else mybir.dt.float32r
BFNP = ml_dtypes.bfloat16 if USE_BF16 else np.float32

KE = EMBED // P         # 8  k-chunks for EMBED contraction
KH = HID // P           # 4  k-chunks for HID contraction
KA = ATT // P           # 2  k-chunks for ATT contraction
KV = SIMG // P          # 96 k-chunks for the visual-context matmul
KI = IMG // P           # 16 k-chunks for separate-image projection
KBH = BSH // P          # 3  k-chunks for history averaging
NHT = HID // P          # 4  hid tiles
NAT = ATT // P          # 2  att tiles

WVG = 6                 # W_vis chunks per DMA group
NWVG = KV // WVG        # 16 groups (2 consumed per mm1 batch row)
WSG = 4                 # W_sep chunks per DMA group
NWSG = KI // WSG        # 4 groups


def build_nc():
    nc = bacc.Bacc(None)

    # ---- DRAM I/O; everything host-prepacked to partition-major blocks ----
    d_reps = nc.dram_tensor("repsp", [BC, P, KE, L], BF, kind="ExternalInput")
    d_wvis = nc.dram_tensor("wvisp", [NWVG, P, WVG, HID], BF, kind="ExternalInput")
    d_vct = nc.dram_tensor("vctp", [P, KV, BC], BF, kind="ExternalInput")
    d_wemb = nc.dram_tensor("wembp", [P, KE, HID], BF, kind="ExternalInput")
    d_wmm = nc.dram_tensor("wmmp", [P, 2 * KH, HID], BF, kind="ExternalInput")
    d_wa1 = nc.dram_tensor("wa1p", [P, KH, ATT], BF, kind="ExternalInput")
    d_wa2 = nc.dram_tensor("wa2p", [P, KA], BF, kind="ExternalInput")
    d_wsep = nc.dram_tensor("wsepp", [NWSG, P, WSG, HID], BF, kind="ExternalInput")
    d_sepT = nc.dram_tensor("sepTp", [P, KI, BS], BF, kind="ExternalInput")
    d_histf = nc.dram_tensor("histfp", [P, KBH, EMBED], BF, kind="ExternalInput")
    d_validW = nc.dram_tensor("validWp", [P, KBH, BS], BF, kind="ExternalInput")
    d_bembc = nc.dram_tensor("bembc", [P, NHT], FP, kind="ExternalInput")
    d_bmmc = nc.dram_tensor("bmmc", [P, NHT], FP, kind="ExternalInput")
    d_ba1c = nc.dram_tensor("ba1c", [P, NAT], FP, kind="ExternalInput")
    d_bvis = nc.dram_tensor("bvis_row", [1, HID], BF, kind="ExternalInput")
    d_bsep = nc.dram_tensor("bsep_row", [1, HID], BF, kind="ExternalInput")
    d_bembr = nc.dram_tensor("bemb_row", [1, HID], BF, kind="ExternalInput")
    d_ones = nc.dram_tensor("ones_row", [1, P], BF, kind="ExternalInput")
    d_ident = nc.dram_tensor("ident", [P, P], FP, kind="ExternalInput")
    d_mask = nc.dram_tensor("mask_row", [1, BC * L], FP, kind="ExternalInput")
    d_hh = nc.dram_tensor("hh_col", [BS, 1], FP, kind="ExternalInput")
    d_diagT = nc.dram_tensor("diagT", [BC, BS], BF, kind="ExternalInput")
    d_out = nc.dram_tensor("out", [BS, 1], FP, kind="ExternalOutput")

    AFT = mybir.ActivationFunctionType
    AX = mybir.AxisListType
    ALU = mybir.AluOpType

    with ExitStack() as ctx:
        tc = ctx.enter_context(tile.TileContext(nc))
        wres = ctx.enter_context(tc.tile_pool(name="wres", bufs=1))
        repsp = ctx.enter_context(tc.tile_pool(name="repsp", bufs=3))
        wvp = ctx.enter_context(tc.tile_pool(name="wvp", bufs=5))
        wsp = ctx.enter_context(tc.tile_pool(name="wsp", bufs=4))
        mm2p = ctx.enter_context(tc.tile_pool(name="mm2p", bufs=12))
        atthp = ctx.enter_context(tc.tile_pool(name="atthp", bufs=4))
        erp = ctx.enter_context(tc.tile_pool(name="erp", bufs=3))
        wbp = ctx.enter_context(tc.tile_pool(name="wbp", bufs=2))
        scrp = ctx.enter_context(tc.tile_pool(name="scrp", bufs=2))
        smp = ctx.enter_context(tc.tile_pool(name="smp", bufs=2))
        psA = ctx.enter_context(tc.tile_pool(name="psA", bufs=5, space="PSUM"))
        psB = ctx.enter_context(tc.tile_pool(name="psB", bufs=2, space="PSUM"))
        psC = ctx.enter_context(tc.tile_pool(name="psC", bufs=1, space="PSUM"))

        def wtile(shape, tag, dt=FP):
            return wres.tile(shape, dt, tag=tag, name=tag)

        def load(dst, src):
            nc.sync.dma_start(out=dst, in_=src)

        def body():
            # ---- phase-A residents, issued in consumption order ----
            wemb = wtile([P, KE, HID], "wemb", BF)
            load(wemb, d_wemb[:, :, :])
            bembc = wtile([P, NHT], "bembc")
            load(bembc, d_bembc[:, :])

            rt = {}

            def load_reps(b):
                t = repsp.tile([P, KE, L], BF, tag="reps", name="rt")
                load(t, d_reps[b])
                rt[b] = t

            wv_tiles = {}

            def load_wv(g):
                t = wvp.tile([P, WVG, HID], BF, tag="wv", name="wv")
                load(t, d_wvis[g])
                wv_tiles[g] = t

            load_reps(0)
            vct = wtile([P, KV, BC], "vct", BF)
            load(vct, d_vct[:, :, :])
            load_wv(0)
            load_wv(1)
            load_reps(1)

            # ---- small constants (negligible bytes, issued early) ----
            ones = wtile([1, P], "ones", BF)
            load(ones, d_ones[:, :])
            ident = wtile([P, P], "ident")
            load(ident, d_ident[:, :])
            bvis_sb = wtile([1, HID], "bvis", BF)
            load(bvis_sb, d_bvis[:, :])
            bsep_sb = wtile([1, HID], "bsep", BF)
            load(bsep_sb, d_bsep[:, :])
            bembr_sb = wtile([1, HID], "bembr", BF)
            load(bembr_sb, d_bembr[:, :])
            bmmc_sb = wtile([P, NHT], "bmmc")
            load(bmmc_sb, d_bmmc[:, :])
            ba1c_sb = wtile([P, NAT], "ba1c")
            load(ba1c_sb, d_ba1c[:, :])
            wa2_sb = wtile([P, KA], "wa2", BF)
            load(wa2_sb, d_wa2[:, :])
            mask_sb = wtile([1, BC * L], "mask")
            load(mask_sb, d_mask[:, :])
            hh_sb = wtile([BS, 1], "hh")
            load(hh_sb, d_hh[:, :])
            diagT_sb = wtile([BC, BS], "diagT", BF)
            load(diagT_sb, d_diagT[:, :])

            # ---- medium phase-B weights (arrive long before needed) ----
            wmm = wtile([P, 2 * KH, HID], "wmm", BF)
            load(wmm, d_wmm[:, :, :])
            wa1 = wtile([P, KH, ATT], "wa1", BF)
            load(wa1, d_wa1[:, :, :])
            sepT_sb = wtile([P, KI, BS], "sepT", BF)
            load(sepT_sb, d_sepT[:, :, :])
            histf_sb = wtile([P, KBH, EMBED], "histf", BF)
            load(histf_sb, d_histf[:, :, :])
            validW_sb = wtile([P, KBH, BS], "validW", BF)
            load(validW_sb, d_validW[:, :, :])

            vc_psum = psB.tile([BC, HID], FP, tag="B", name="vc_psum")
            mm1_sb = {}

            def emit_vc_group(g):
                wv = wv_tiles[g]
                for j in range(WVG):
                    k = g * WVG + j
                    nc.tensor.matmul(vc_psum[:, :], vct[:, k, :], wv[:, j, :],
                                     start=(k == 0), stop=False)

            def emit_mm1(b):
                for h in range(NHT):
                    ps = psA.tile([P, L], FP, tag="A", name="mm1ps")
                    for k in range(KE):
                        nc.tensor.matmul(
                            ps[:, :],
                            wemb[:, k, h * P:(h + 1) * P],
                            rt[b][:, k, :],
                            start=(k == 0), stop=(k == KE - 1))
                    t = wtile([P, L], f"mm1_{b}_{h}", BF)
                    nc.scalar.activation(t, ps[:, :], AFT.Relu,
                                         bias=bembc[:, h:h + 1])
                    mm1_sb[(b, h)] = t

            # ---- phase A: mm1 for all b, vc chunk-stream interleaved ----
            wsg_loaded = 0
            ws_tiles = {}
            for b in range(BC):
                if b + 2 < BC:
                    load_reps(b + 2)
                if 2 * b + 2 < NWVG:
                    load_wv(2 * b + 2)
                if 2 * b + 3 < NWVG:
                    load_wv(2 * b + 3)
                if b >= 6 and wsg_loaded < NWSG:
                    # stream W_sep groups in during the phase-A tail
                    for _ in range(2):
                        t = wsp.tile([P, WSG, HID], BF, tag="ws", name="ws")
                        load(t, d_wsep[wsg_loaded])
                        ws_tiles[wsg_loaded] = t
                        wsg_loaded += 1
                emit_mm1(b)
                emit_vc_group(2 * b)
                emit_vc_group(2 * b + 1)

            # vc bias + relu -> ctx rows [8, 512] (fp32 for PE transpose)
            nc.tensor.matmul(vc_psum[:, :], ones[:, :BC], bvis_sb[:, :],
                             start=False, stop=True)
            ctx_sb = wtile([BC, HID], "ctx_sb")
            nc.scalar.activation(ctx_sb, vc_psum[:, :], AFT.Relu)

            # ---- separate images projection: sep[48, 512] ----
            sep_ps = psB.tile([BS, HID], FP, tag="B", name="sep_ps")
            for g in range(NWSG):
                ws = ws_tiles[g]
                for j in range(WSG):
                    k = g * WSG + j
                    nc.tensor.matmul(sep_ps[:, :], sepT_sb[:, k, :],
                                     ws[:, j, :],
                                     start=(k == 0), stop=False)
            nc.tensor.matmul(sep_ps[:, :], ones[:, :BS], bsep_sb[:, :],
                             start=False, stop=True)

            # transpose ctx -> ctxT [512, 8] (PE, fp32 in, bf16 out)
            ctxT_sb = [wtile([P, BC], f"ctxT{h}", BF) for h in range(NHT)]
            for h in range(NHT):
                tp = psC.tile([P, BC], FP, tag="C", name="ctxT_ps")
                nc.tensor.transpose(tp[:, :], ctx_sb[:, h * P:(h + 1) * P],
                                    ident[:BC, :BC])
                nc.scalar.activation(ctxT_sb[h], tp[:, :], AFT.Identity)

            # ctxmmb[h2] = W_mm_bot.T @ ctxT + b_mm   [128, 8] per hid2 tile
            ctxmmb_sb = [wtile([P, BC], f"ctxmmb{h}") for h in range(NHT)]
            for h2 in range(NHT):
                ps = psC.tile([P, BC], FP, tag="C", name="ctxmm_ps")
                for k in range(KH):
                    nc.tensor.matmul(ps[:, :],
                                     wmm[:, KH + k, h2 * P:(h2 + 1) * P],
                                     ctxT_sb[k][:, :],
                                     start=(k == 0), stop=(k == KH - 1))
                nc.scalar.activation(ctxmmb_sb[h2], ps[:, :], AFT.Identity,
                                     bias=bmmc_sb[:, h2:h2 + 1])

            sep_sb = wtile([BS, HID], "sep_sb")
            nc.vector.tensor_copy(sep_sb, sep_ps[:, :])

            # ---- history: havgT[e, 48] via valid-weight matmul ----
            havgT_sb = [wtile([P, BS], f"havgT{e}", BF) for e in range(KE)]
            for e in range(KE):
                ps = psB.tile([P, BS], FP, tag="B", name="havg_ps")
                for k in range(KBH):
                    nc.tensor.matmul(ps[:, :],
                                     histf_sb[:, k, e * P:(e + 1) * P],
                                     validW_sb[:, k, :],
                                     start=(k == 0), stop=(k == KBH - 1))
                nc.scalar.activation(havgT_sb[e], ps[:, :], AFT.Identity)

            # hist_add[48, 512] = relu(havg @ W_emb + b_emb)
            ha_ps = psB.tile([BS, HID], FP, tag="B", name="ha_ps")
            for e in range(KE):
                nc.tensor.matmul(ha_ps[:, :], havgT_sb[e][:, :],
                                 wemb[:, e, :],
                                 start=(e == 0), stop=False)
            nc.tensor.matmul(ha_ps[:, :], ones[:, :BS], bembr_sb[:, :],
                             start=False, stop=True)
            hadd_sb = wtile([BS, HID], "hadd_sb")
            nc.scalar.activation(hadd_sb, ha_ps[:, :], AFT.Relu)

            # sep_final = sep + hh * hist_add
            sepfin_sb = wtile([BS, HID], "sepfin_sb")
            nc.vector.tensor_scalar_mul(sepfin_sb, hadd_sb, hh_sb)
            nc.vector.tensor_add(sepfin_sb, sepfin_sb, sep_sb)

            # ---- phase B: per-b mm2 -> mm3 -> scores, softmax one b behind
            attT_sb = [wtile([P, BC], f"attT{h}") for h in range(NHT)]
            mm2_tiles = {}
            wrow_tiles = {}

            def emit_attended(b):
                # broadcast normalized weight row to [128, L] via PE product
                wb_ps = psA.tile([P, L], FP, tag="A", name="wbps")
                nc.tensor.matmul(wb_ps[:, :], ones[:, :], wrow_tiles[b][:, :],
                                 start=True, stop=True)
                wb_sb = wbp.tile([P, L], BF, tag="wb", name="wb_sb")
                nc.scalar.activation(wb_sb, wb_ps[:, :], AFT.Identity)
                for h2 in range(NHT):
                    tmp = scrp.tile([P, L], FP, tag="scr", name="scr")
                    if USE_BF16:
                        nc.vector.tensor_mul(tmp, mm2_tiles[b][h2], wb_sb)
                    else:
                        nc.vector.tensor_mul(tmp,
                                             mm2_tiles[b][h2].bitcast(FP),
                                             wb_sb.bitcast(FP))
                    nc.vector.reduce_sum(attT_sb[h2][:, b:b + 1], tmp,
                                         axis=AX.X)

            for b in range(BC):
                # mm2T[b]: [hid2, L] = relu(Wmm_top.T @ mm1T[b] + ctxmm[:, b])
                mm2t = []
                for h2 in range(NHT):
                    ps = psA.tile([P, L], FP, tag="A", name="mm2ps")
                    for k in range(KH):
                        nc.tensor.matmul(ps[:, :],
                                         wmm[:, k, h2 * P:(h2 + 1) * P],
                                         mm1_sb[(b, k)][:, :],
                                         start=(k == 0), stop=(k == KH - 1))
                    t = mm2p.tile([P, L], BF, tag="mm2", name="mm2t")
                    nc.scalar.activation(t, ps[:, :], AFT.Relu,
                                         bias=ctxmmb_sb[h2][:, b:b + 1])
                    mm2t.append(t)
                mm2_tiles[b] = mm2t
                # mm3: atthT [att, L] = tanh(W_a1.T @ mm2T + b_a1)
                atth = []
                for a in range(NAT):
                    ps = psA.tile([P, L], FP, tag="A", name="mm3ps")
                    for k in range(KH):
                        nc.tensor.matmul(ps[:, :],
                                         wa1[:, k, a * P:(a + 1) * P],
                                         mm2t[k][:, :],
                                         start=(k == 0), stop=(k == KH - 1))
                    t = atthp.tile([P, L], BF, tag="atth", name="atht")
                    nc.scalar.activation(t, ps[:, :], AFT.Tanh,
                                         bias=ba1c_sb[:, a:a + 1])
                    atth.append(t)
                # scores row [1, L] = W_a2.T @ atthT
                sc_ps = psC.tile([1, L], FP, tag="C", name="scps")
                for k in range(KA):
                    nc.tensor.matmul(sc_ps[:, :], wa2_sb[:, k:k + 1],
                                     atth[k][:, :],
                                     start=(k == 0), stop=(k == KA - 1))
                # mask add (mask holds -1e30 for masked, + b_a2), then
                # softmax over L: e = exp(x - max), normalized by 1/esum
                arow = smp.tile([1, L], FP, tag="arow", name="arow")
                nc.vector.tensor_add(arow, sc_ps[:, :],
                                     mask_sb[:, b * L:(b + 1) * L])
                negmax = smp.tile([1, 1], FP, tag="negmax", name="negmax")
                nc.vector.reduce_max(negmax, arow, axis=AX.X, negate=True)
                esum = smp.tile([1, 1], FP, tag="esum", name="esum")
                nc.scalar.activation(arow, arow, AFT.Exp, bias=negmax,
                                     accum_out=esum)
                rec = smp.tile([1, 1], FP, tag="rec", name="rec")
                nc.vector.reciprocal(rec, esum)
                wrow = erp.tile([1, L], BF, tag="wrow", name="wrow")
                nc.scalar.activation(wrow, arow, AFT.Copy, scale=rec)
                wrow_tiles[b] = wrow
                if b > 0:
                    emit_attended(b - 1)
            emit_attended(BC - 1)

            # ---- assemble attended rows [8, 512] ----
            attrows_sb = wtile([BC, HID], "attrows", BF)
            for h in range(NHT):
                tp = psB.tile([BC, P], FP, tag="B", name="attrow_ps")
                nc.tensor.transpose(tp[:, :], attT_sb[h][:, :], ident[:, :])
                nc.scalar.activation(attrows_sb[:, h * P:(h + 1) * P],
                                     tp[:, :], AFT.Identity)

            # broadcast to [48, 512]: diagT.T @ attrows, then final dot
            ab_ps = psB.tile([BS, HID], FP, tag="B", name="ab_ps")
            nc.tensor.matmul(ab_ps[:, :], diagT_sb[:, :], attrows_sb[:, :],
                             start=True, stop=True)
            prod = wtile([BS, HID], "prod")
            nc.vector.tensor_mul(prod, sepfin_sb, ab_ps[:, :])
            out_sb = wtile([BS, 1], "out_sb")
            nc.vector.reduce_sum(out_sb, prod, axis=AX.X)
            nc.sync.dma_start(out=d_out[:, :], in_=out_sb)

        body()

    nc.compile()
    return nc


_NC_CACHE = None


def kernel(reps, separate_imgs, visual_context, masks, hist, hist_len,
           W_vis, b_vis, W_emb, b_emb, W_mm, b_mm, W_sep, b_sep,
           W_a1, b_a1, W_a2, b_a2):
    global _NC_CACHE
    f32 = np.float32

    def pack(a, K, W):
        """[K*128, W] -> [128, K, W] partition-major bf16."""
        a = np.asarray(a, f32).astype(BFNP)
        return np.ascontiguousarray(
            a.reshape(K, P, W).transpose(1, 0, 2))

    reps = np.asarray(reps, f32)
    separate_imgs = np.asarray(separate_imgs, f32)
    visual_context = np.asarray(visual_context, f32)
    hist = np.asarray(hist, f32)
    hist_len = np.asarray(hist_len, np.int32)
    masks = np.asarray(masks)

    # reps_p[b, p, k, l] = reps[b, l, k*128 + p]
    reps_p = np.ascontiguousarray(
        reps.astype(BFNP).reshape(B, L, KE, P).transpose(0, 3, 2, 1))
    mask_row = np.where(masks[:, :, 0], f32(-1e30), f32(0.0)) + f32(b_a2[0])

    shared = {
        "wvisp": np.ascontiguousarray(
            np.asarray(W_vis, f32).astype(BFNP)
            .reshape(NWVG, WVG, P, HID).transpose(0, 2, 1, 3)),
        "wembp": pack(W_emb, KE, HID),
        "wmmp": pack(W_mm, 2 * KH, HID),
        "wa1p": pack(W_a1, KH, ATT),
        "wa2p": np.ascontiguousarray(
            np.asarray(W_a2, f32).astype(BFNP).reshape(KA, P).T),
        "wsepp": np.ascontiguousarray(
            np.asarray(W_sep, f32).astype(BFNP)
            .reshape(NWSG, WSG, P, HID).transpose(0, 2, 1, 3)),
        "bembc": np.ascontiguousarray(
            np.asarray(b_emb, f32).reshape(NHT, P).T),
        "bmmc": np.ascontiguousarray(
            np.asarray(b_mm, f32).reshape(NHT, P).T),
        "ba1c": np.ascontiguousarray(
            np.asarray(b_a1, f32).reshape(NAT, P).T),
        "bvis_row": np.asarray(b_vis, f32).astype(BFNP).reshape(1, HID),
        "bsep_row": np.asarray(b_sep, f32).astype(BFNP).reshape(1, HID),
        "bemb_row": np.asarray(b_emb, f32).astype(BFNP).reshape(1, HID),
        "ones_row": np.ones((1, P), BFNP),
        "ident": np.eye(P, dtype=f32),
        "diagT": np.repeat(np.eye(BC, dtype=f32), S, axis=1)
                   .reshape(BC, BS).astype(BFNP),
    }

    in_maps = []
    for c in range(NCORES):
        sl = slice(c * BC, (c + 1) * BC)
        hl = hist_len[sl].reshape(BS)                            # [48]
        hvalid = (np.arange(H)[None, :] < hl[:, None]).astype(f32)
        hvalid /= np.maximum(hl, 1).astype(f32)[:, None]         # [48, H]
        validW = np.zeros((BSH, BS), f32)
        for bs in range(BS):
            validW[bs * H:(bs + 1) * H, bs] = hvalid[bs]
        m = {
            "repsp": reps_p[sl],
            "vctp": np.ascontiguousarray(
                visual_context[sl].astype(BFNP)
                .reshape(BC, KV, P).transpose(2, 1, 0)),
            "sepTp": np.ascontiguousarray(
                separate_imgs[sl].astype(BFNP)
                .reshape(BS, KI, P).transpose(2, 1, 0)),
            "histfp": pack(hist[sl].reshape(BSH, EMBED), KBH, EMBED),
            "validWp": pack(validW, KBH, BS),
            "mask_row": np.ascontiguousarray(mask_row[sl]).reshape(1, BC * L),
            "hh_col": (hl > 0).astype(f32).reshape(BS, 1),
        }
        m.update(shared)
        in_maps.append(m)

    if _NC_CACHE is None:
        _NC_CACHE = build_nc()
    res = run_bass_kernel_spmd(_NC_CACHE, in_maps, list(range(NCORES)))
    out = np.concatenate([r["out"].reshape(BC, S, 1) for r in res.results],
                         axis=0)
    return out.astype(f32)


if __name__ == "__main__":
    pass
